# revision 3
# baseline (speedup 1.0000x reference)
"""Causal self-attention (single head) on 8 TRN2 NeuronCores.

Sharding: data-parallel over batch (4) x query-interleave (2).
Core c handles batch b = c//2 and 8 query blocks of 128 chosen so that
the two cores of a batch have equal causal work:
  j=0 -> blocks [0,3,4,7,8,11,12,15],  j=1 -> blocks [1,2,5,6,9,10,13,14]
Slot t of each core processes its query block against the first
256*(t+1) keys (a superset of the causal requirement for both cores'
blocks in that slot), with the exact causal mask applied from per-core
query-position data.  All cores run one identical SPMD program; only
input data differs.

Compute dtype: bf16 matmuls (f32 PSUM accumulate), f32 softmax stats.
"""

import os
from contextlib import ExitStack

import numpy as np
import ml_dtypes

B, S, D = 4, 2048, 1024
P = 128
ND = D // P  # 8 d (contraction) chunks
NE = D // P  # 8 output-feature chunks
NSK = S // P  # 16 key chunks
NQB = 8  # query blocks per core
SQH = NQB * P  # 1024 queries per core
J_BLOCKS = (
    [0, 3, 4, 7, 8, 11, 12, 15],
    [1, 2, 5, 6, 9, 10, 13, 14],
)
COVS = [256 * (t + 1) for t in range(NQB)]  # key coverage per slot
SCALE = 1.0 / np.sqrt(np.float32(D))  # 1/32
NEG_BIG = -1.0e30

_NC = None


def _score_tiles(cov):
    """(offset, width) score tiles of <=512 columns covering [0, cov)."""
    tiles = [(off, 512) for off in range(0, cov - cov % 512, 512)]
    if cov % 512:
        tiles.append((cov - cov % 512, cov % 512))
    return tiles


def _emit(nc, tc, dr, out_d):
    import concourse.bass as bass  # noqa: F401
    from concourse import mybir

    BF = mybir.dt.bfloat16
    F32 = mybir.dt.float32
    AF = mybir.ActivationFunctionType
    Alu = mybir.AluOpType
    X = mybir.AxisListType.X

    with ExitStack() as ctx:
        const = ctx.enter_context(tc.tile_pool(name="const", bufs=1))
        iota = const.tile([P, S], F32)
        nc.sync.dma_start(iota[:], dr["iota"])
        qpos = const.tile([P, NQB], F32)
        nc.sync.dma_start(qpos[:], dr["qpos"])
        bq = const.tile([P, NE], F32)
        nc.sync.dma_start(bq[:], dr["bq"])
        bk = const.tile([P, NE], F32)
        nc.sync.dma_start(bk[:], dr["bk"])
        bv = const.tile([P, D], F32)
        nc.sync.dma_start(bv[:], dr["bv"])
        bo = const.tile([P, D], F32)
        nc.sync.dma_start(bo[:], dr["bo"])
        ident = const.tile([P, P], BF)
        nc.sync.dma_start(ident[:], dr["ident"])

        # persistent activation storage
        qt_pool = ctx.enter_context(tc.tile_pool(name="qt", bufs=NE))
        kt_pool = ctx.enter_context(tc.tile_pool(name="kt", bufs=NE))
        v_pool = ctx.enter_context(tc.tile_pool(name="v", bufs=NSK))
        wot_pool = ctx.enter_context(tc.tile_pool(name="wot", bufs=ND))
        QT, KT, V, WOT = [], [], [], []

        # ---------------- phase A: projections ----------------
        with ExitStack() as actx:
            xqt_pool = actx.enter_context(tc.tile_pool(name="xqt", bufs=ND))
            xt_pool = actx.enter_context(tc.tile_pool(name="xt", bufs=ND))
            wvt_pool = actx.enter_context(tc.tile_pool(name="wvt", bufs=ND))
            wstr_pool = actx.enter_context(tc.tile_pool(name="wstr", bufs=8))
            psp = actx.enter_context(tc.tile_pool(name="psp", bufs=8, space="PSUM"))

            XQ, XT, WV = [], [], []
            for d in range(ND):
                xq = xqt_pool.tile([P, SQH], BF)
                nc.sync.dma_start(xq[:], dr["xqT"][d])
                XQ.append(xq)
            for d in range(ND):
                xt = xt_pool.tile([P, S], BF)
                nc.sync.dma_start(xt[:], dr["xT"][d])
                XT.append(xt)
            for d in range(ND):
                wv = wvt_pool.tile([P, D], BF)
                nc.sync.dma_start(wv[:], dr["wvT"][d])
                WV.append(wv)

            # QT[e] = (Wq @ x_q^T)[e-chunk] + bq  -> [128 e, 1024 q] bf16
            for e in range(NE):
                qts = qt_pool.tile([P, SQH], BF)
                pss = [psp.tile([P, 512], F32, tag="ps", name="ps") for _ in range(2)]
                for d in range(ND):
                    w = wstr_pool.tile([P, P], BF, tag="wq")
                    nc.sync.dma_start(w[:], dr["wqT"][e * ND + d])
                    for nt in range(2):
                        nc.tensor.matmul(
                            pss[nt][:],
                            w[:],
                            XQ[d][:, nt * 512 : (nt + 1) * 512],
                            start=(d == 0),
                            stop=(d == ND - 1),
                        )
                for nt in range(2):
                    nc.scalar.activation(
                        qts[:, nt * 512 : (nt + 1) * 512],
                        pss[nt][:],
                        AF.Identity,
                        bias=bq[:, e : e + 1],
                        scale=1.0,
                    )
                QT.append(qts)

            # KT[e] = (Wk @ x^T)[e-chunk] + bk -> [128 e, 2048 k] bf16
            for e in range(NE):
                kts = kt_pool.tile([P, S], BF)
                pss = [psp.tile([P, 512], F32, tag="ps", name="ps") for _ in range(4)]
                for d in range(ND):
                    w = wstr_pool.tile([P, P], BF, tag="wk")
                    nc.sync.dma_start(w[:], dr["wkT"][e * ND + d])
                    for nt in range(4):
                        nc.tensor.matmul(
                            pss[nt][:],
                            w[:],
                            XT[d][:, nt * 512 : (nt + 1) * 512],
                            start=(d == 0),
                            stop=(d == ND - 1),
                        )
                for nt in range(4):
                    nc.scalar.activation(
                        kts[:, nt * 512 : (nt + 1) * 512],
                        pss[nt][:],
                        AF.Identity,
                        bias=bk[:, e : e + 1],
                        scale=1.0,
                    )
                KT.append(kts)

            # V[s] = (x @ Wv^T)[s-chunk] + bv -> [128 s, 1024 e] bf16
            for s in range(NSK):
                vs = v_pool.tile([P, D], BF)
                pss = [psp.tile([P, 512], F32, tag="ps", name="ps") for _ in range(2)]
                for d in range(ND):
                    for nt in range(2):
                        nc.tensor.matmul(
                            pss[nt][:],
                            XT[d][:, s * P : (s + 1) * P],
                            WV[d][:, nt * 512 : (nt + 1) * 512],
                            start=(d == 0),
                            stop=(d == ND - 1),
                        )
                for nt in range(2):
                    nc.vector.tensor_tensor(
                        vs[:, nt * 512 : (nt + 1) * 512],
                        pss[nt][:],
                        bv[:, nt * 512 : (nt + 1) * 512],
                        op=Alu.add,
                    )
                V.append(vs)

        for d in range(ND):
            wo = wot_pool.tile([P, D], BF)
            nc.sync.dma_start(wo[:], dr["woT"][d])
            WOT.append(wo)

        # ---------------- phase B: attention + output projection ----------------
        with ExitStack() as bctx:
            sp = bctx.enter_context(tc.tile_pool(name="s_sb", bufs=2))
            wp = bctx.enter_context(tc.tile_pool(name="w_sb", bufs=2))
            wtp = bctx.enter_context(tc.tile_pool(name="wt_sb", bufs=2))
            attp = bctx.enter_context(tc.tile_pool(name="att", bufs=2))
            atttp = bctx.enter_context(tc.tile_pool(name="attT", bufs=2))
            outp = bctx.enter_context(tc.tile_pool(name="out_sb", bufs=2))
            stat = bctx.enter_context(tc.tile_pool(name="stat", bufs=3))
            ps_s = bctx.enter_context(tc.tile_pool(name="ps_s", bufs=2, space="PSUM"))
            ps_t = bctx.enter_context(tc.tile_pool(name="ps_t", bufs=2, space="PSUM"))
            ps_a = bctx.enter_context(tc.tile_pool(name="ps_a", bufs=2, space="PSUM"))
            ps_o = bctx.enter_context(tc.tile_pool(name="ps_o", bufs=2, space="PSUM"))

            def emit_scores(t):
                cov = COVS[t]
                s_sb = sp.tile([P, cov], F32, tag="s")
                # mask term: (k > q) * -1e30, written into s_sb
                nc.vector.tensor_scalar(
                    s_sb[:],
                    iota[:, :cov],
                    qpos[:, t : t + 1],
                    NEG_BIG,
                    op0=Alu.is_gt,
                    op1=Alu.mult,
                )
                for off, wdt in _score_tiles(cov):
                    ps = ps_s.tile([P, wdt], F32, tag="ps_s")
                    for e in range(NE):
                        nc.tensor.matmul(
                            ps[:],
                            QT[e][:, t * P : (t + 1) * P],
                            KT[e][:, off : off + wdt],
                            start=(e == 0),
                            stop=(e == NE - 1),
                        )
                    nc.vector.tensor_tensor(
                        s_sb[:, off : off + wdt],
                        ps[:],
                        s_sb[:, off : off + wdt],
                        op=Alu.add,
                    )
                negm = stat.tile([P, 1], F32, tag="negm")
                nc.vector.tensor_reduce(
                    negm[:], s_sb[:], axis=X, op=Alu.max, negate=True
                )
                negm32 = stat.tile([P, 1], F32, tag="negm32")
                nc.vector.tensor_scalar_mul(negm32[:], negm[:], float(SCALE))
                w_sb = wp.tile([P, cov], BF, tag="w")
                lsum = stat.tile([P, 1], F32, tag="lsum")
                nc.scalar.activation(
                    w_sb[:],
                    s_sb[:],
                    AF.Exp,
                    bias=negm32[:],
                    scale=float(SCALE),
                    accum_out=lsum[:],
                )
                rinv = stat.tile([P, 1], F32, tag="rinv")
                nc.vector.reciprocal(rinv[:], lsum[:])
                return {"t": t, "w": w_sb, "rinv": rinv}

            def emit_tail(st):
                t = st["t"]
                cov = COVS[t]
                K = cov // P
                w_sb, rinv = st["w"], st["rinv"]
                wT = wtp.tile([P, cov], BF, tag="wt")
                for k in range(K):
                    pt = ps_t.tile([P, P], BF, tag="pt")
                    nc.tensor.transpose(pt[:], w_sb[:, k * P : (k + 1) * P], ident[:])
                    nc.vector.tensor_copy(wT[:, k * P : (k + 1) * P], pt[:])
                att = attp.tile([P, D], BF, tag="att")
                for nt in range(2):
                    pa = ps_a.tile([P, 512], F32, tag="pa")
                    for k in range(K):
                        nc.tensor.matmul(
                            pa[:],
                            wT[:, k * P : (k + 1) * P],
                            V[k][:, nt * 512 : (nt + 1) * 512],
                            start=(k == 0),
                            stop=(k == K - 1),
                        )
                    nc.scalar.activation(
                        att[:, nt * 512 : (nt + 1) * 512],
                        pa[:],
                        AF.Copy,
                        bias=0.0,
                        scale=rinv[:],
                    )
                attT = atttp.tile([P, D], BF, tag="attT")
                for d in range(ND):
                    pt = ps_t.tile([P, P], BF, tag="pt")
                    nc.tensor.transpose(pt[:], att[:, d * P : (d + 1) * P], ident[:])
                    nc.vector.tensor_copy(attT[:, d * P : (d + 1) * P], pt[:])
                outsb = outp.tile([P, D], F32, tag="o")
                for nt in range(2):
                    po = ps_o.tile([P, 512], F32, tag="po")
                    for d in range(ND):
                        nc.tensor.matmul(
                            po[:],
                            attT[:, d * P : (d + 1) * P],
                            WOT[d][:, nt * 512 : (nt + 1) * 512],
                            start=(d == 0),
                            stop=(d == ND - 1),
                        )
                    nc.vector.tensor_tensor(
                        outsb[:, nt * 512 : (nt + 1) * 512],
                        po[:],
                        bo[:, nt * 512 : (nt + 1) * 512],
                        op=Alu.add,
                    )
                nc.sync.dma_start(out_d[t], outsb[:])

            prev = None
            for t in range(NQB):
                cur = emit_scores(t)
                if prev is not None:
                    emit_tail(prev)
                prev = cur
            emit_tail(prev)


def build_nc():
    """Build + compile the SPMD Bass program (cached)."""
    global _NC
    if _NC is not None:
        return _NC
    from concourse import bacc, mybir
    import concourse.tile as tile

    BF = mybir.dt.bfloat16
    F32 = mybir.dt.float32

    nc = bacc.Bacc(
        "TRN2", target_bir_lowering=False, debug=False, enable_asserts=False
    )
    dr = {}

    def din(name, shape, dt):
        dr[name] = nc.dram_tensor(name, shape, dt, kind="ExternalInput").ap()

    din("xT", (ND, P, S), BF)
    din("xqT", (ND, P, SQH), BF)
    din("wqT", (NE * ND, P, P), BF)
    din("wkT", (NE * ND, P, P), BF)
    din("wvT", (ND, P, D), BF)
    din("woT", (ND, P, D), BF)
    din("qpos", (P, NQB), F32)
    din("iota", (P, S), F32)
    din("bq", (P, NE), F32)
    din("bk", (P, NE), F32)
    din("bv", (P, D), F32)
    din("bo", (P, D), F32)
    din("ident", (P, P), BF)
    out_d = nc.dram_tensor("out_c", (NQB, P, D), F32, kind="ExternalOutput").ap()

    with tile.TileContext(nc) as tc:
        _emit(nc, tc, dr, out_d)
    nc.compile()
    _NC = nc
    return nc


def make_in_maps(x, Wq, bq, Wk, bk, Wv, bv, Wo, bo):
    """Host-side sharding: per-core input dicts (bf16 compute operands)."""
    bf16 = ml_dtypes.bfloat16
    f32 = np.float32

    def chunk64(WT):  # [1024,1024] -> (64,128,128), index e*8+d
        return (
            np.ascontiguousarray(
                WT.reshape(ND, P, NE, P).transpose(2, 0, 1, 3)
            ).reshape(NE * ND, P, P)
        ).astype(bf16)

    WqT = np.ascontiguousarray(Wq.T)
    WkT = np.ascontiguousarray(Wk.T)
    WvT = np.ascontiguousarray(Wv.T)
    WoT = np.ascontiguousarray(Wo.T)
    wq_c = chunk64(WqT)
    wk_c = chunk64(WkT)
    wv_c = WvT.reshape(ND, P, D).astype(bf16)
    wo_c = WoT.reshape(ND, P, D).astype(bf16)
    iota = np.broadcast_to(
        np.arange(S, dtype=f32), (P, S)
    ).copy()
    bq_t = np.ascontiguousarray(bq.reshape(NE, P).T).astype(f32)
    bk_t = np.ascontiguousarray(bk.reshape(NE, P).T).astype(f32)
    bv_b = np.broadcast_to(bv.astype(f32), (P, D)).copy()
    bo_b = np.broadcast_to(bo.astype(f32), (P, D)).copy()
    ident = np.eye(P, dtype=bf16)

    in_maps = []
    for c in range(8):
        b, j = c // 2, c % 2
        blocks = J_BLOCKS[j]
        xTb = np.ascontiguousarray(x[b].T)  # [D, S]
        qcols = np.concatenate(
            [np.arange(P * g, P * (g + 1)) for g in blocks]
        )
        xq = np.ascontiguousarray(xTb[:, qcols])  # [D, 1024]
        qpos = (np.array(blocks, dtype=f32) * P)[None, :] + np.arange(
            P, dtype=f32
        )[:, None]
        in_maps.append(
            {
                "xT": xTb.reshape(ND, P, S).astype(bf16),
                "xqT": xq.reshape(ND, P, SQH).astype(bf16),
                "wqT": wq_c,
                "wkT": wk_c,
                "wvT": wv_c,
                "woT": wo_c,
                "qpos": np.ascontiguousarray(qpos),
                "iota": iota,
                "bq": bq_t,
                "bk": bk_t,
                "bv": bv_b,
                "bo": bo_b,
                "ident": ident,
            }
        )
    return in_maps


def assemble_out(results):
    out = np.empty((B, S, D), dtype=np.float32)
    for c in range(8):
        b, j = c // 2, c % 2
        blocks = J_BLOCKS[j]
        oc = results[c]["out_c"]  # (8, 128, 1024)
        for t, g in enumerate(blocks):
            out[b, P * g : P * (g + 1), :] = oc[t]
    return out


def kernel(x, Wq, bq, Wk, bk, Wv, bv, Wo, bo):
    from concourse.bass_utils import run_bass_kernel_spmd

    nc = build_nc()
    in_maps = make_in_maps(x, Wq, bq, Wk, bk, Wv, bv, Wo, bo)
    res = run_bass_kernel_spmd(nc, in_maps, core_ids=list(range(8)))
    return assemble_out(res.results)


# revision 26
# speedup vs baseline: 16672.6965x; 16672.6965x over previous
"""Causal self-attention (single head) on 8 TRN2 NeuronCores.

Sharding: data-parallel over batch (4) x query-interleave (2).
Core c handles batch b = c//2 and 8 query blocks of 128 chosen so that
the two cores of a batch have equal causal work:
  j=0 -> blocks [0,3,4,7,8,11,12,15],  j=1 -> blocks [1,2,5,6,9,10,13,14]
Slot t of each core processes its query block against the first
256*(t+1) keys (a superset of the causal requirement for both cores'
blocks in that slot), with the exact causal mask applied from per-core
query-position data.  All cores run one identical SPMD program; only
input data differs.

Compute dtype: bf16 matmuls (f32 PSUM accumulate), f32 softmax stats.
"""

from contextlib import ExitStack

import numpy as np
import ml_dtypes

B, S, D = 4, 2048, 1024
P = 128
ND = D // P  # 8 d (contraction) chunks
NE = D // P  # 8 output-feature chunks
NSK = S // P  # 16 key chunks
NQB = 8  # query blocks per core
SQH = NQB * P  # 1024 queries per core
J_BLOCKS = (
    [0, 3, 4, 7, 8, 11, 12, 15],
    [1, 2, 5, 6, 9, 10, 13, 14],
)
COVS = [256 * (t + 1) for t in range(NQB)]  # key coverage per slot
SCALE = 1.0 / np.sqrt(np.float32(D))  # 1/32
NEG_BIG = -1.0e30
CPAK = NQB + 2 * NE + S + 2 * D  # packed f32 consts width

_NC = None


def _score_tiles(cov):
    """(offset, width) score tiles of <=512 columns covering [0, cov)."""
    tiles = [(off, 512) for off in range(0, cov - cov % 512, 512)]
    if cov % 512:
        tiles.append((cov - cov % 512, cov % 512))
    return tiles


def _emit(nc, tc, dr, out_d):
    import concourse.bass as bass  # noqa: F401
    from concourse import mybir

    BF = mybir.dt.bfloat16
    F32 = mybir.dt.float32
    AF = mybir.ActivationFunctionType
    Alu = mybir.AluOpType
    X = mybir.AxisListType.X

    with ExitStack() as ctx:
        const = ctx.enter_context(tc.tile_pool(name="const", bufs=1))
        # packed f32 consts: tiny slice first (needed by the first
        # evictions), bulky slice (kpos/bv/bo) loaded later off the
        # critical path
        cpak = const.tile([P, CPAK], F32)
        nc.sync.dma_start(cpak[:, : NQB + 2 * NE], dr["cpak"][:, : NQB + 2 * NE])
        qpos = cpak[:, 0:NQB]
        bq = cpak[:, NQB : NQB + NE]
        bk = cpak[:, NQB + NE : NQB + 2 * NE]
        kpos = cpak[:, NQB + 2 * NE : NQB + 2 * NE + S]
        bv = cpak[:, NQB + 2 * NE + S : NQB + 2 * NE + S + D]
        bo = cpak[:, NQB + 2 * NE + S + D : NQB + 2 * NE + S + 2 * D]
        ident = const.tile([P, P], BF)
        nc.sync.dma_start(ident[:], dr["ident"])
        # persistent activation storage
        qt_pool = ctx.enter_context(tc.tile_pool(name="qt", bufs=NE))
        kt_pool = ctx.enter_context(tc.tile_pool(name="kt", bufs=NE))
        v_pool = ctx.enter_context(tc.tile_pool(name="v", bufs=NSK))
        wot_pool = ctx.enter_context(tc.tile_pool(name="wot", bufs=ND))
        QT, KT, V, WOT = [], [], [], []

        # ---------------- phase A: projections ----------------
        # Keys are host-permuted so this core's query block t sits at
        # columns [256t, 256t+128) of xT — Q is projected straight from
        # xT slices (no separate x_q input).  Weight chunks arrive one
        # batched DMA per e-group (8 chunks) to keep the per-trigger
        # sync-engine cost (~0.7us each) off the critical path.
        with ExitStack() as actx:
            xt_pool = actx.enter_context(tc.tile_pool(name="xt", bufs=ND))
            wvt_pool = actx.enter_context(tc.tile_pool(name="wvt", bufs=ND))
            wstr_pool = actx.enter_context(tc.tile_pool(name="wstr", bufs=4))
            psp = actx.enter_context(tc.tile_pool(name="psp", bufs=8, space="PSUM"))

            XT = [None] * ND
            WV = []

            def load_wgroup(key, e, tag, bufs=None, eng=None):
                wg = wstr_pool.tile(
                    [P, ND * P], BF, tag=tag, name=tag, bufs=bufs
                )
                (eng or nc.scalar).dma_start(wg[:], dr[key][e])
                return wg

            # x stream split across both rings, interleaved with the wq
            # groups so neither the first matmul nor the d-loop starves:
            #   sync ring:   xT[0], xT[2], xT[4], xT[6]
            #   scalar ring: wq[0], xT[1], wq[1], xT[3], ... wq[4..7]
            for d in (0, 2, 4, 6):
                xt = xt_pool.tile([P, S], BF, name="xt")
                nc.sync.dma_start(xt[:], dr["xT"][d])
                XT[d] = xt
            WQG = []
            for e in range(4):
                WQG.append(load_wgroup("wqT", e, "wq", bufs=NE))
                d = 2 * e + 1
                xt = xt_pool.tile([P, S], BF, name="xt")
                nc.scalar.dma_start(xt[:], dr["xT"][d])
                XT[d] = xt
            for e in range(4, NE):
                # sync ring is idle after the xT evens — balance the load
                WQG.append(load_wgroup("wqT", e, "wq", bufs=NE, eng=nc.sync))

            # QT[e] = (Wq @ x_q^T)[e-chunk] + bq  -> [128 e, 1024 q] bf16
            for e in range(NE):
                qts = qt_pool.tile([P, SQH], BF)
                wg = WQG[e]
                pss = [psp.tile([P, 512], F32, tag="ps", name="ps") for _ in range(2)]
                for d in range(ND):
                    for nt in range(2):
                        # strided rhs: first 128 of each 256-col slot pair
                        # (this core's queries) -> N=512 in one matmul
                        rhs = XT[d][
                            :, nt * 1024 : (nt + 1) * 1024
                        ].rearrange("p (t c) -> p t c", t=4)[:, :, 0:P]
                        nc.tensor.matmul(
                            pss[nt][:],
                            wg[:, d * P : (d + 1) * P],
                            rhs,
                            start=(d == 0),
                            stop=(d == ND - 1),
                        )
                for nt in range(2):
                    nc.scalar.activation(
                        qts[:, nt * 512 : (nt + 1) * 512],
                        pss[nt][:],
                        AF.Identity,
                        bias=bq[:, e : e + 1],
                        scale=1.0,
                    )
                QT.append(qts)

            # KT[e] = (Wk @ x^T)[e-chunk] + bk -> [128 e, 2048 k] bf16
            for e in range(NE):
                kts = kt_pool.tile([P, S], BF)
                wg = load_wgroup("wkT", e, "wk")
                pss = [psp.tile([P, 512], F32, tag="ps", name="ps") for _ in range(4)]
                for d in range(ND):
                    for nt in range(4):
                        nc.tensor.matmul(
                            pss[nt][:],
                            wg[:, d * P : (d + 1) * P],
                            XT[d][:, nt * 512 : (nt + 1) * 512],
                            start=(d == 0),
                            stop=(d == ND - 1),
                        )
                for nt in range(4):
                    nc.scalar.activation(
                        kts[:, nt * 512 : (nt + 1) * 512],
                        pss[nt][:],
                        AF.Identity,
                        bias=bk[:, e : e + 1],
                        scale=1.0,
                    )
                KT.append(kts)

            for d in range(ND):
                wv = wvt_pool.tile([P, D], BF)
                nc.scalar.dma_start(wv[:], dr["wvT"][d])
                WV.append(wv)
            nc.sync.dma_start(
                cpak[:, NQB + 2 * NE :], dr["cpak"][:, NQB + 2 * NE :]
            )

            # V[s] = (x @ Wv^T)[s-chunk] + bv -> [128 s, 1024 e] bf16
            for s in range(NSK):
                vs = v_pool.tile([P, D], BF)
                pss = [psp.tile([P, 512], F32, tag="ps", name="ps") for _ in range(2)]
                for d in range(ND):
                    for nt in range(2):
                        nc.tensor.matmul(
                            pss[nt][:],
                            XT[d][:, s * P : (s + 1) * P],
                            WV[d][:, nt * 512 : (nt + 1) * 512],
                            start=(d == 0),
                            stop=(d == ND - 1),
                        )
                for nt in range(2):
                    nc.vector.tensor_tensor(
                        vs[:, nt * 512 : (nt + 1) * 512],
                        pss[nt][:],
                        bv[:, nt * 512 : (nt + 1) * 512],
                        op=Alu.add,
                    )
                V.append(vs)

        for d in range(ND):
            wo = wot_pool.tile([P, D], BF)
            nc.scalar.dma_start(wo[:], dr["woT"][d])
            WOT.append(wo)

        # ---------------- phase B: attention + output projection ----------------
        # 3-stage software pipeline over query blocks (big blocks first):
        #   S(t): score matmuls + mask + softmax + DMA-transpose of weights
        #   A(t): attended matmuls + rinv-scaled evict + DMA-transpose
        #   O(t): output projection + bias + store
        # PE program order S(i);A(i-1);O(i-2) keeps the PE dense while
        # softmax/DMA latencies of a block hide under the next block's
        # score matmuls.
        with ExitStack() as bctx:
            sp = bctx.enter_context(tc.tile_pool(name="s_sb", bufs=2))
            wp = bctx.enter_context(tc.tile_pool(name="w_sb", bufs=2))
            wtp = bctx.enter_context(tc.tile_pool(name="wt_sb", bufs=3))
            attp = bctx.enter_context(tc.tile_pool(name="att", bufs=2))
            atttp = bctx.enter_context(tc.tile_pool(name="attT", bufs=3))
            outp = bctx.enter_context(tc.tile_pool(name="out_sb", bufs=2))
            stat = bctx.enter_context(tc.tile_pool(name="stat", bufs=3))
            ps_s = bctx.enter_context(tc.tile_pool(name="ps_s", bufs=2, space="PSUM"))
            ps_t = bctx.enter_context(tc.tile_pool(name="ps_t", bufs=2, space="PSUM"))
            ps_a = bctx.enter_context(tc.tile_pool(name="ps_a", bufs=2, space="PSUM"))
            ps_o = bctx.enter_context(tc.tile_pool(name="ps_o", bufs=2, space="PSUM"))

            def emit_scores(t):
                cov = COVS[t]
                s_sb = sp.tile([P, cov], F32, tag="s")
                # mask term: (k > q) * -1e30, written into s_sb
                nc.vector.tensor_scalar(
                    s_sb[:],
                    kpos[:, :cov],
                    qpos[:, t : t + 1],
                    NEG_BIG,
                    op0=Alu.is_gt,
                    op1=Alu.mult,
                )
                for off, wdt in _score_tiles(cov):
                    ps = ps_s.tile([P, wdt], F32, tag="ps_s")
                    for e in range(NE):
                        nc.tensor.matmul(
                            ps[:],
                            QT[e][:, t * P : (t + 1) * P],
                            KT[e][:, off : off + wdt],
                            start=(e == 0),
                            stop=(e == NE - 1),
                        )
                    nc.vector.tensor_tensor(
                        s_sb[:, off : off + wdt],
                        ps[:],
                        s_sb[:, off : off + wdt],
                        op=Alu.add,
                    )
                negm = stat.tile([P, 1], F32, tag="negm")
                nc.vector.tensor_reduce(
                    negm[:], s_sb[:], axis=X, op=Alu.max, negate=True
                )
                negm32 = stat.tile([P, 1], F32, tag="negm32")
                nc.vector.tensor_scalar_mul(negm32[:], negm[:], float(SCALE))
                w_sb = wp.tile([P, cov], BF, tag="w")
                lsum = stat.tile([P, 1], F32, tag="lsum")
                nc.scalar.activation(
                    w_sb[:],
                    s_sb[:],
                    AF.Exp,
                    bias=negm32[:],
                    scale=float(SCALE),
                    accum_out=lsum[:],
                )
                rinv = stat.tile([P, 1], F32, tag="rinv")
                nc.vector.reciprocal(rinv[:], lsum[:])
                # weight transposes on PE (matmul transpose mode)
                K = cov // P
                wT = wtp.tile([P, cov], BF, tag="wt")
                for k in range(K):
                    pt = ps_t.tile([P, P], BF, tag="pt")
                    nc.tensor.transpose(pt[:], w_sb[:, k * P : (k + 1) * P], ident[:])
                    nc.vector.tensor_copy(wT[:, k * P : (k + 1) * P], pt[:])
                return {"t": t, "wT": wT, "rinv": rinv}

            def emit_attend(st):
                t = st["t"]
                cov = COVS[t]
                K = cov // P
                wT, rinv = st["wT"], st["rinv"]
                att = attp.tile([P, D], BF, tag="att")
                for nt in range(2):
                    pa = ps_a.tile([P, 512], F32, tag="pa")
                    for k in range(K):
                        nc.tensor.matmul(
                            pa[:],
                            wT[:, k * P : (k + 1) * P],
                            V[k][:, nt * 512 : (nt + 1) * 512],
                            start=(k == 0),
                            stop=(k == K - 1),
                        )
                    nc.scalar.activation(
                        att[:, nt * 512 : (nt + 1) * 512],
                        pa[:],
                        AF.Copy,
                        bias=0.0,
                        scale=rinv[:],
                    )
                attT = atttp.tile([P, D], BF, tag="attT")
                for d in range(ND):
                    pt = ps_t.tile([P, P], BF, tag="pt")
                    nc.tensor.transpose(pt[:], att[:, d * P : (d + 1) * P], ident[:])
                    nc.vector.tensor_copy(attT[:, d * P : (d + 1) * P], pt[:])
                st["attT"] = attT

            def emit_out(st):
                t = st["t"]
                attT = st["attT"]
                outsb = outp.tile([P, D], F32, tag="o")
                for nt in range(2):
                    po = ps_o.tile([P, 512], F32, tag="po")
                    for d in range(ND):
                        nc.tensor.matmul(
                            po[:],
                            attT[:, d * P : (d + 1) * P],
                            WOT[d][:, nt * 512 : (nt + 1) * 512],
                            start=(d == 0),
                            stop=(d == ND - 1),
                        )
                    nc.vector.tensor_tensor(
                        outsb[:, nt * 512 : (nt + 1) * 512],
                        po[:],
                        bo[:, nt * 512 : (nt + 1) * 512],
                        op=Alu.add,
                    )
                    nc.sync.dma_start(
                        out_d[t][:, nt * 512 : (nt + 1) * 512],
                        outsb[:, nt * 512 : (nt + 1) * 512],
                    )

            order = list(range(NQB - 1, -1, -1))  # big blocks first
            states = []
            for i, t in enumerate(order):
                states.append(emit_scores(t))
                if i >= 1:
                    emit_attend(states[i - 1])
                if i >= 2:
                    emit_out(states[i - 2])
            emit_attend(states[-1])
            emit_out(states[-2])
            emit_out(states[-1])


def build_nc():
    """Build + compile the SPMD Bass program (cached)."""
    global _NC
    if _NC is not None:
        return _NC
    from concourse import bacc, mybir
    import concourse.tile as tile

    BF = mybir.dt.bfloat16
    F32 = mybir.dt.float32

    nc = bacc.Bacc(
        "TRN2", target_bir_lowering=False, debug=False, enable_asserts=False
    )
    dr = {}

    def din(name, shape, dt):
        dr[name] = nc.dram_tensor(name, shape, dt, kind="ExternalInput").ap()

    din("xT", (ND, P, S), BF)
    din("wqT", (NE, P, ND * P), BF)
    din("wkT", (NE, P, ND * P), BF)
    din("wvT", (ND, P, D), BF)
    din("woT", (ND, P, D), BF)
    din("ident", (P, P), BF)
    din("cpak", (P, CPAK), F32)
    out_d = nc.dram_tensor("out_c", (NQB, P, D), F32, kind="ExternalOutput").ap()

    with tile.TileContext(nc) as tc:
        _emit(nc, tc, dr, out_d)
    nc.compile()
    _NC = nc
    return nc


def make_in_maps(x, Wq, bq, Wk, bk, Wv, bv, Wo, bo):
    """Host-side sharding: per-core input dicts (bf16 compute operands)."""
    bf16 = ml_dtypes.bfloat16
    f32 = np.float32

    def chunkg(WT):  # [1024,1024] -> (NE,128,ND*128): [e][p][d*128+c]
        return (
            np.ascontiguousarray(
                WT.reshape(ND, P, NE, P).transpose(2, 1, 0, 3)
            ).reshape(NE, P, ND * P)
        ).astype(bf16)

    WqT = np.ascontiguousarray(Wq.T)
    WkT = np.ascontiguousarray(Wk.T)
    WvT = np.ascontiguousarray(Wv.T)
    WoT = np.ascontiguousarray(Wo.T)
    wq_c = chunkg(WqT)
    wk_c = chunkg(WkT)
    wv_c = WvT.reshape(ND, P, D).astype(bf16)
    wo_c = WoT.reshape(ND, P, D).astype(bf16)
    bq_t = np.ascontiguousarray(bq.reshape(NE, P).T).astype(f32)
    bk_t = np.ascontiguousarray(bk.reshape(NE, P).T).astype(f32)
    bv_b = np.broadcast_to(bv.astype(f32), (P, D))
    bo_b = np.broadcast_to(bo.astype(f32), (P, D))
    ident = np.eye(P, dtype=bf16)

    in_maps = []
    for c in range(8):
        b, j = c // 2, c % 2
        blocks = J_BLOCKS[j]
        other = J_BLOCKS[1 - j]
        # key permutation: slot t holds [my block t | peer block t], so
        # this core's queries are columns [256t, 256t+128) and the first
        # 256(t+1) columns cover every true key <= any query in slot t
        perm = np.concatenate(
            [
                np.r_[P * blocks[t] : P * (blocks[t] + 1),
                      P * other[t] : P * (other[t] + 1)]
                for t in range(NQB)
            ]
        )
        xTb = np.ascontiguousarray(x[b].T[:, perm])  # [D, S] permuted keys
        qpos = (np.array(blocks, dtype=f32) * P)[None, :] + np.arange(
            P, dtype=f32
        )[:, None]
        kpos = np.broadcast_to(perm.astype(f32), (P, S))
        cpak = np.concatenate([qpos, bq_t, bk_t, kpos, bv_b, bo_b], axis=1)
        assert cpak.shape == (P, CPAK)
        in_maps.append(
            {
                "xT": xTb.reshape(ND, P, S).astype(bf16),
                "wqT": wq_c,
                "wkT": wk_c,
                "wvT": wv_c,
                "woT": wo_c,
                "cpak": np.ascontiguousarray(cpak.astype(f32)),
                "ident": ident,
            }
        )
    return in_maps


def assemble_out(results):
    out = np.empty((B, S, D), dtype=np.float32)
    for c in range(8):
        b, j = c // 2, c % 2
        blocks = J_BLOCKS[j]
        oc = results[c]["out_c"]  # (8, 128, 1024)
        for t, g in enumerate(blocks):
            out[b, P * g : P * (g + 1), :] = oc[t]
    return out


def kernel(x, Wq, bq, Wk, bk, Wv, bv, Wo, bo):
    from concourse.bass_utils import run_bass_kernel_spmd

    nc = build_nc()
    in_maps = make_in_maps(x, Wq, bq, Wk, bk, Wv, bv, Wo, bo)
    res = run_bass_kernel_spmd(nc, in_maps, core_ids=list(range(8)))
    return assemble_out(res.results)


# revision 29
# speedup vs baseline: 23994.3482x; 1.4391x over previous
"""Causal self-attention (single head) on 8 TRN2 NeuronCores.

Sharding: data-parallel over batch (4) x query-interleave (2).
Core c handles batch b = c//2 and 8 query blocks of 128 chosen so that
the two cores of a batch have equal causal work:
  j=0 -> blocks [0,3,4,7,8,11,12,15],  j=1 -> blocks [1,2,5,6,9,10,13,14]
Slot t of each core processes its query block against the first
256*(t+1) keys (a superset of the causal requirement for both cores'
blocks in that slot), with the exact causal mask applied from per-core
query-position data.  All cores run one identical SPMD program; only
input data differs.

Compute dtype: bf16 matmuls (f32 PSUM accumulate), f32 softmax stats.
"""

from contextlib import ExitStack

import numpy as np
import ml_dtypes

B, S, D = 4, 2048, 1024
P = 128
ND = D // P  # 8 d (contraction) chunks
NE = D // P  # 8 output-feature chunks
NSK = S // P  # 16 key chunks
NQB = 8  # query blocks per core
SQH = NQB * P  # 1024 queries per core
J_BLOCKS = (
    [0, 3, 4, 7, 8, 11, 12, 15],
    [1, 2, 5, 6, 9, 10, 13, 14],
)
COVS = [256 * (t + 1) for t in range(NQB)]  # key coverage per slot
SCALE = 1.0 / np.sqrt(np.float32(D))  # 1/32
NEG_BIG = -1.0e30
CPAK = NQB + 2 * NE + S + 2 * D  # packed f32 consts width

_NC = None


def _score_tiles(cov):
    """(offset, width) score tiles of <=512 columns covering [0, cov)."""
    tiles = [(off, 512) for off in range(0, cov - cov % 512, 512)]
    if cov % 512:
        tiles.append((cov - cov % 512, cov % 512))
    return tiles


def _emit(nc, tc, dr, out_d):
    import concourse.bass as bass  # noqa: F401
    from concourse import mybir

    BF = mybir.dt.bfloat16
    F32 = mybir.dt.float32
    AF = mybir.ActivationFunctionType
    Alu = mybir.AluOpType
    X = mybir.AxisListType.X

    with ExitStack() as ctx:
        const = ctx.enter_context(tc.tile_pool(name="const", bufs=1))
        # packed f32 consts: tiny slice first (needed by the first
        # evictions), bulky slice (kpos/bv/bo) loaded later off the
        # critical path
        cpak = const.tile([P, CPAK], F32)
        nc.sync.dma_start(cpak[:, : NQB + 2 * NE], dr["cpak"][:, : NQB + 2 * NE])
        qpos = cpak[:, 0:NQB]
        bq = cpak[:, NQB : NQB + NE]
        bk = cpak[:, NQB + NE : NQB + 2 * NE]
        kpos = cpak[:, NQB + 2 * NE : NQB + 2 * NE + S]
        bv = cpak[:, NQB + 2 * NE + S : NQB + 2 * NE + S + D]
        bo = cpak[:, NQB + 2 * NE + S + D : NQB + 2 * NE + S + 2 * D]
        ident = const.tile([P, P], BF)
        nc.sync.dma_start(ident[:], dr["ident"])
        # persistent activation storage
        qt_pool = ctx.enter_context(tc.tile_pool(name="qt", bufs=NE))
        v_pool = ctx.enter_context(tc.tile_pool(name="v", bufs=NSK))
        xt_pool = ctx.enter_context(tc.tile_pool(name="xt", bufs=ND))
        QT, V = [], []

        # ---------------- phase A: projections ----------------
        # Keys are host-permuted so this core's query block t sits at
        # columns [256t, 256t+128) of xT — Q is projected straight from
        # xT slices (no separate x_q input).  Weight chunks arrive one
        # batched DMA per e-group (8 chunks) to keep the per-trigger
        # sync-engine cost (~0.7us each) off the critical path.
        with ExitStack() as actx:
            wvt_pool = actx.enter_context(tc.tile_pool(name="wvt", bufs=ND))
            wstr_pool = actx.enter_context(tc.tile_pool(name="wstr", bufs=4))
            psp = actx.enter_context(tc.tile_pool(name="psp", bufs=8, space="PSUM"))

            XT = [None] * ND
            WV = []

            def load_wgroup(key, e, tag, bufs=None, eng=None):
                wg = wstr_pool.tile(
                    [P, ND * P], BF, tag=tag, name=tag, bufs=bufs
                )
                (eng or nc.scalar).dma_start(wg[:], dr[key][e])
                return wg

            # x stream split across both rings, interleaved with the wq
            # groups so neither the first matmul nor the d-loop starves:
            #   sync ring:   xT[0], xT[2], xT[4], xT[6]
            #   scalar ring: wq[0], xT[1], wq[1], xT[3], ... wq[4..7]
            for d in (0, 2, 4, 6):
                xt = xt_pool.tile([P, S], BF, name="xt")
                nc.sync.dma_start(xt[:], dr["xT"][d])
                XT[d] = xt
            WQG = []
            for e in range(4):
                WQG.append(load_wgroup("wqT", e, "wq", bufs=NE))
                d = 2 * e + 1
                xt = xt_pool.tile([P, S], BF, name="xt")
                nc.scalar.dma_start(xt[:], dr["xT"][d])
                XT[d] = xt
            for e in range(4, NE):
                # sync ring is idle after the xT evens — balance the load
                WQG.append(load_wgroup("wqT", e, "wq", bufs=NE, eng=nc.sync))

            # QT[e] = (Wq @ x_q^T)[e-chunk] + bq  -> [128 e, 1024 q] bf16
            for e in range(NE):
                qts = qt_pool.tile([P, SQH], BF)
                wg = WQG[e]
                pss = [psp.tile([P, 512], F32, tag="ps", name="ps") for _ in range(2)]
                for d in range(ND):
                    for nt in range(2):
                        # strided rhs: first 128 of each 256-col slot pair
                        # (this core's queries) -> N=512 in one matmul
                        rhs = XT[d][
                            :, nt * 1024 : (nt + 1) * 1024
                        ].rearrange("p (t c) -> p t c", t=4)[:, :, 0:P]
                        nc.tensor.matmul(
                            pss[nt][:],
                            wg[:, d * P : (d + 1) * P],
                            rhs,
                            start=(d == 0),
                            stop=(d == ND - 1),
                        )
                for nt in range(2):
                    nc.scalar.activation(
                        qts[:, nt * 512 : (nt + 1) * 512],
                        pss[nt][:],
                        AF.Identity,
                        bias=bq[:, e : e + 1],
                        scale=1.0,
                    )
                QT.append(qts)

            for d in range(ND):
                wv = wvt_pool.tile([P, D], BF)
                nc.scalar.dma_start(wv[:], dr["wvT"][d])
                WV.append(wv)
            nc.sync.dma_start(
                cpak[:, NQB + 2 * NE :], dr["cpak"][:, NQB + 2 * NE :]
            )

            # V[s] = (x @ Wv^T)[s-chunk] + bv -> [128 s, 1024 e] bf16
            for s in range(NSK):
                vs = v_pool.tile([P, D], BF)
                pss = [psp.tile([P, 512], F32, tag="ps", name="ps") for _ in range(2)]
                for d in range(ND):
                    for nt in range(2):
                        nc.tensor.matmul(
                            pss[nt][:],
                            XT[d][:, s * P : (s + 1) * P],
                            WV[d][:, nt * 512 : (nt + 1) * 512],
                            start=(d == 0),
                            stop=(d == ND - 1),
                        )
                for nt in range(2):
                    nc.vector.tensor_tensor(
                        vs[:, nt * 512 : (nt + 1) * 512],
                        pss[nt][:],
                        bv[:, nt * 512 : (nt + 1) * 512],
                        op=Alu.add,
                    )
                V.append(vs)

        # ---------------- phase B: attention + output projection ----------------
        # 3-stage software pipeline over query blocks (big blocks first):
        #   S(t): score matmuls + mask + softmax + DMA-transpose of weights
        #   A(t): attended matmuls + rinv-scaled evict + DMA-transpose
        #   O(t): output projection + bias + store
        # PE program order S(i);A(i-1);O(i-2) keeps the PE dense while
        # softmax/DMA latencies of a block hide under the next block's
        # score matmuls.
        with ExitStack() as bctx:
            sp = bctx.enter_context(tc.tile_pool(name="s_sb", bufs=2))
            wp = bctx.enter_context(tc.tile_pool(name="w_sb", bufs=2))
            wtp = bctx.enter_context(tc.tile_pool(name="wt_sb", bufs=3))
            outp = bctx.enter_context(tc.tile_pool(name="out_sb", bufs=2))
            stat = bctx.enter_context(tc.tile_pool(name="stat", bufs=3))
            ps_s = bctx.enter_context(tc.tile_pool(name="ps_s", bufs=2, space="PSUM"))
            ps_t = bctx.enter_context(tc.tile_pool(name="ps_t", bufs=2, space="PSUM"))
            ps_a = bctx.enter_context(tc.tile_pool(name="ps_a", bufs=3, space="PSUM"))

            def emit_scores(t):
                cov = COVS[t]
                s_sb = sp.tile([P, cov], F32, tag="s")
                # mask term: (k > q) * -1e30, written into s_sb
                nc.vector.tensor_scalar(
                    s_sb[:],
                    kpos[:, :cov],
                    qpos[:, t : t + 1],
                    NEG_BIG,
                    op0=Alu.is_gt,
                    op1=Alu.mult,
                )
                for off, wdt in _score_tiles(cov):
                    ps = ps_s.tile([P, wdt], F32, tag="ps_s")
                    for e in range(NE):
                        nc.tensor.matmul(
                            ps[:],
                            QT[e][:, t * P : (t + 1) * P],
                            XT[e][:, off : off + wdt],
                            start=(e == 0),
                            stop=(e == NE - 1),
                        )
                    nc.vector.tensor_tensor(
                        s_sb[:, off : off + wdt],
                        ps[:],
                        s_sb[:, off : off + wdt],
                        op=Alu.add,
                    )
                negm = stat.tile([P, 1], F32, tag="negm")
                nc.vector.tensor_reduce(
                    negm[:], s_sb[:], axis=X, op=Alu.max, negate=True
                )
                negm32 = stat.tile([P, 1], F32, tag="negm32")
                nc.vector.tensor_scalar_mul(negm32[:], negm[:], float(SCALE))
                w_sb = wp.tile([P, cov], BF, tag="w")
                lsum = stat.tile([P, 1], F32, tag="lsum")
                nc.scalar.activation(
                    w_sb[:],
                    s_sb[:],
                    AF.Exp,
                    bias=negm32[:],
                    scale=float(SCALE),
                    accum_out=lsum[:],
                )
                rinv = stat.tile([P, 1], F32, tag="rinv")
                nc.vector.reciprocal(rinv[:], lsum[:])
                # weight transposes on PE (matmul transpose mode)
                K = cov // P
                wT = wtp.tile([P, cov], BF, tag="wt")
                for k in range(K):
                    pt = ps_t.tile([P, P], BF, tag="pt")
                    nc.tensor.transpose(pt[:], w_sb[:, k * P : (k + 1) * P], ident[:])
                    nc.vector.tensor_copy(wT[:, k * P : (k + 1) * P], pt[:])
                return {"t": t, "wT": wT, "rinv": rinv}

            def emit_attend(st):
                t = st["t"]
                cov = COVS[t]
                K = cov // P
                wT, rinv = st["wT"], st["rinv"]
                outsb = outp.tile([P, D], F32, tag="o")
                for nt in range(2):
                    pa = ps_a.tile([P, 512], F32, tag="pa")
                    for k in range(K):
                        nc.tensor.matmul(
                            pa[:],
                            wT[:, k * P : (k + 1) * P],
                            V[k][:, nt * 512 : (nt + 1) * 512],
                            start=(k == 0),
                            stop=(k == K - 1),
                        )
                    # out = psum * rinv (softmax normalize) then + bvo
                    nc.scalar.activation(
                        outsb[:, nt * 512 : (nt + 1) * 512],
                        pa[:],
                        AF.Copy,
                        bias=0.0,
                        scale=rinv[:],
                    )
                    nc.vector.tensor_tensor(
                        outsb[:, nt * 512 : (nt + 1) * 512],
                        outsb[:, nt * 512 : (nt + 1) * 512],
                        bo[:, nt * 512 : (nt + 1) * 512],
                        op=Alu.add,
                    )
                    nc.sync.dma_start(
                        out_d[t][:, nt * 512 : (nt + 1) * 512],
                        outsb[:, nt * 512 : (nt + 1) * 512],
                    )

            order = list(range(NQB - 1, -1, -1))  # big blocks first
            states = []
            for i, t in enumerate(order):
                states.append(emit_scores(t))
                if i >= 1:
                    emit_attend(states[i - 1])
            emit_attend(states[-1])


def build_nc():
    """Build + compile the SPMD Bass program (cached)."""
    global _NC
    if _NC is not None:
        return _NC
    from concourse import bacc, mybir
    import concourse.tile as tile

    BF = mybir.dt.bfloat16
    F32 = mybir.dt.float32

    nc = bacc.Bacc(
        "TRN2", target_bir_lowering=False, debug=False, enable_asserts=False
    )
    dr = {}

    def din(name, shape, dt):
        dr[name] = nc.dram_tensor(name, shape, dt, kind="ExternalInput").ap()

    din("xT", (ND, P, S), BF)
    din("wqT", (NE, P, ND * P), BF)
    din("wvT", (ND, P, D), BF)
    din("ident", (P, P), BF)
    din("cpak", (P, CPAK), F32)
    out_d = nc.dram_tensor("out_c", (NQB, P, D), F32, kind="ExternalOutput").ap()

    with tile.TileContext(nc) as tc:
        _emit(nc, tc, dr, out_d)
    nc.compile()
    _NC = nc
    return nc


def make_in_maps(x, Wq, bq, Wk, bk, Wv, bv, Wo, bo):
    """Host-side sharding: per-core input dicts (bf16 compute operands)."""
    bf16 = ml_dtypes.bfloat16
    f32 = np.float32

    def chunkg(WT):  # [1024,1024] -> (NE,128,ND*128): [e][p][d*128+c]
        return (
            np.ascontiguousarray(
                WT.reshape(ND, P, NE, P).transpose(2, 1, 0, 3)
            ).reshape(NE, P, ND * P)
        ).astype(bf16)

    # host-fused weights (f32 GEMMs, exact up to fp32):
    #   scores = (x Wq^T)(x Wk^T)^T = x (Wq^T Wk) x^T       -> Wqk
    #   out    = softmax(..) (x Wv^T) Wo^T = softmax(..) x (Wo Wv)^T
    # so K and the output projection never materialize on-chip.
    # Requires bq = bk = 0 (guaranteed by the problem spec).
    Wqk = Wq.T.astype(np.float32) @ Wk.astype(np.float32)  # [d1, d2]
    Wvo = Wo.astype(np.float32) @ Wv.astype(np.float32)  # [e, d]
    wq_c = chunkg(Wqk)
    wv_c = np.ascontiguousarray(Wvo.T).reshape(ND, P, D).astype(bf16)
    bvo = Wo.astype(np.float32) @ bv.astype(np.float32) + bo.astype(np.float32)
    bq_t = np.ascontiguousarray(bq.reshape(NE, P).T).astype(f32)
    bk_t = np.ascontiguousarray(bk.reshape(NE, P).T).astype(f32)
    bv_b = np.zeros((P, D), f32)  # bv folded into bvo
    bo_b = np.broadcast_to(bvo, (P, D))
    ident = np.eye(P, dtype=bf16)

    in_maps = []
    for c in range(8):
        b, j = c // 2, c % 2
        blocks = J_BLOCKS[j]
        other = J_BLOCKS[1 - j]
        # key permutation: slot t holds [my block t | peer block t], so
        # this core's queries are columns [256t, 256t+128) and the first
        # 256(t+1) columns cover every true key <= any query in slot t
        perm = np.concatenate(
            [
                np.r_[P * blocks[t] : P * (blocks[t] + 1),
                      P * other[t] : P * (other[t] + 1)]
                for t in range(NQB)
            ]
        )
        xTb = np.ascontiguousarray(x[b].T[:, perm])  # [D, S] permuted keys
        qpos = (np.array(blocks, dtype=f32) * P)[None, :] + np.arange(
            P, dtype=f32
        )[:, None]
        kpos = np.broadcast_to(perm.astype(f32), (P, S))
        cpak = np.concatenate([qpos, bq_t, bk_t, kpos, bv_b, bo_b], axis=1)
        assert cpak.shape == (P, CPAK)
        in_maps.append(
            {
                "xT": xTb.reshape(ND, P, S).astype(bf16),
                "wqT": wq_c,
                "wvT": wv_c,
                "cpak": np.ascontiguousarray(cpak.astype(f32)),
                "ident": ident,
            }
        )
    return in_maps


def assemble_out(results):
    out = np.empty((B, S, D), dtype=np.float32)
    for c in range(8):
        b, j = c // 2, c % 2
        blocks = J_BLOCKS[j]
        oc = results[c]["out_c"]  # (8, 128, 1024)
        for t, g in enumerate(blocks):
            out[b, P * g : P * (g + 1), :] = oc[t]
    return out


def kernel(x, Wq, bq, Wk, bk, Wv, bv, Wo, bo):
    from concourse.bass_utils import run_bass_kernel_spmd

    nc = build_nc()
    in_maps = make_in_maps(x, Wq, bq, Wk, bk, Wv, bv, Wo, bo)
    res = run_bass_kernel_spmd(nc, in_maps, core_ids=list(range(8)))
    return assemble_out(res.results)


# revision 30
# speedup vs baseline: 24737.2449x; 1.0310x over previous
"""Causal self-attention (single head) on 8 TRN2 NeuronCores.

Sharding: data-parallel over batch (4) x query-interleave (2).
Core c handles batch b = c//2 and 8 query blocks of 128 chosen so that
the two cores of a batch have equal causal work:
  j=0 -> blocks [0,3,4,7,8,11,12,15],  j=1 -> blocks [1,2,5,6,9,10,13,14]
Slot t of each core processes its query block against the first
256*(t+1) keys (a superset of the causal requirement for both cores'
blocks in that slot), with the exact causal mask applied from per-core
query-position data.  All cores run one identical SPMD program; only
input data differs.

Compute dtype: bf16 matmuls (f32 PSUM accumulate), f32 softmax stats.
"""

from contextlib import ExitStack

import numpy as np
import ml_dtypes

B, S, D = 4, 2048, 1024
P = 128
ND = D // P  # 8 d (contraction) chunks
NE = D // P  # 8 output-feature chunks
NSK = S // P  # 16 key chunks
NQB = 8  # query blocks per core
SQH = NQB * P  # 1024 queries per core
J_BLOCKS = (
    [0, 3, 4, 7, 8, 11, 12, 15],
    [1, 2, 5, 6, 9, 10, 13, 14],
)
COVS = [256 * (t + 1) for t in range(NQB)]  # key coverage per slot
SCALE = 1.0 / np.sqrt(np.float32(D))  # 1/32
NEG_BIG = -1.0e30
CPAK = NQB + 2 * NE + S + 2 * D  # packed f32 consts width

_NC = None


def _score_tiles(cov):
    """(offset, width) score tiles of <=512 columns covering [0, cov)."""
    tiles = [(off, 512) for off in range(0, cov - cov % 512, 512)]
    if cov % 512:
        tiles.append((cov - cov % 512, cov % 512))
    return tiles


def _emit(nc, tc, dr, out_d):
    import concourse.bass as bass  # noqa: F401
    from concourse import mybir

    BF = mybir.dt.bfloat16
    F32 = mybir.dt.float32
    AF = mybir.ActivationFunctionType
    Alu = mybir.AluOpType
    X = mybir.AxisListType.X

    with ExitStack() as ctx:
        const = ctx.enter_context(tc.tile_pool(name="const", bufs=1))
        # packed f32 consts: tiny slice first (needed by the first
        # evictions), bulky slice (kpos/bv/bo) loaded later off the
        # critical path
        cpak = const.tile([P, CPAK], F32)
        nc.sync.dma_start(cpak[:, : NQB + 2 * NE], dr["cpak"][:, : NQB + 2 * NE])
        qpos = cpak[:, 0:NQB]
        bq = cpak[:, NQB : NQB + NE]
        bk = cpak[:, NQB + NE : NQB + 2 * NE]
        kpos = cpak[:, NQB + 2 * NE : NQB + 2 * NE + S]
        bv = cpak[:, NQB + 2 * NE + S : NQB + 2 * NE + S + D]
        bo = cpak[:, NQB + 2 * NE + S + D : NQB + 2 * NE + S + 2 * D]
        ident = const.tile([P, P], BF)
        nc.sync.dma_start(ident[:], dr["ident"])
        # persistent activation storage
        qt_pool = ctx.enter_context(tc.tile_pool(name="qt", bufs=NE))
        v_pool = ctx.enter_context(tc.tile_pool(name="v", bufs=NSK))
        xt_pool = ctx.enter_context(tc.tile_pool(name="xt", bufs=ND))
        QT, V = [], []

        # ---------------- phase A: projections ----------------
        # Keys are host-permuted so this core's query block t sits at
        # columns [256t, 256t+128) of xT — Q is projected straight from
        # xT slices (no separate x_q input).  Weight chunks arrive one
        # batched DMA per e-group (8 chunks) to keep the per-trigger
        # sync-engine cost (~0.7us each) off the critical path.
        with ExitStack() as actx:
            wvt_pool = actx.enter_context(tc.tile_pool(name="wvt", bufs=ND))
            wstr_pool = actx.enter_context(tc.tile_pool(name="wstr", bufs=4))
            psp = actx.enter_context(tc.tile_pool(name="psp", bufs=8, space="PSUM"))

            XT = [None] * ND
            WV = []

            def load_wgroup(key, e, tag, bufs=None, eng=None):
                wg = wstr_pool.tile(
                    [P, ND * P], BF, tag=tag, name=tag, bufs=bufs
                )
                (eng or nc.scalar).dma_start(wg[:], dr[key][e])
                return wg

            # x stream split across both rings, interleaved with the wq
            # groups so neither the first matmul nor the d-loop starves:
            #   sync ring:   xT[0], xT[2], xT[4], xT[6]
            #   scalar ring: wq[0], xT[1], wq[1], xT[3], ... wq[4..7]
            for d in (0, 4):
                xt = xt_pool.tile([P, S], BF, name="xt")
                nc.sync.dma_start(xt[:], dr["xT"][d])
                XT[d] = xt
            for d in (2, 6):
                xt = xt_pool.tile([P, S], BF, name="xt")
                nc.gpsimd.dma_start(xt[:], dr["xT"][d])
                XT[d] = xt
            WQG = []
            for e in range(4):
                WQG.append(load_wgroup("wqT", e, "wq", bufs=NE))
                d = 2 * e + 1
                xt = xt_pool.tile([P, S], BF, name="xt")
                nc.scalar.dma_start(xt[:], dr["xT"][d])
                XT[d] = xt
            for e in range(4, NE):
                # sync ring is idle after the xT evens — balance the load
                WQG.append(load_wgroup("wqT", e, "wq", bufs=NE, eng=nc.sync))

            # QT[e] = (Wq @ x_q^T)[e-chunk] + bq  -> [128 e, 1024 q] bf16
            for e in range(NE):
                qts = qt_pool.tile([P, SQH], BF)
                wg = WQG[e]
                pss = [psp.tile([P, 512], F32, tag="ps", name="ps") for _ in range(2)]
                for d in range(ND):
                    for nt in range(2):
                        # strided rhs: first 128 of each 256-col slot pair
                        # (this core's queries) -> N=512 in one matmul
                        rhs = XT[d][
                            :, nt * 1024 : (nt + 1) * 1024
                        ].rearrange("p (t c) -> p t c", t=4)[:, :, 0:P]
                        nc.tensor.matmul(
                            pss[nt][:],
                            wg[:, d * P : (d + 1) * P],
                            rhs,
                            start=(d == 0),
                            stop=(d == ND - 1),
                        )
                for nt in range(2):
                    nc.scalar.activation(
                        qts[:, nt * 512 : (nt + 1) * 512],
                        pss[nt][:],
                        AF.Identity,
                        bias=bq[:, e : e + 1],
                        scale=1.0,
                    )
                QT.append(qts)

            for d in range(ND):
                wv = wvt_pool.tile([P, D], BF)
                nc.sync.dma_start(wv[:], dr["wvT"][d])
                WV.append(wv)
            nc.sync.dma_start(
                cpak[:, NQB + 2 * NE :], dr["cpak"][:, NQB + 2 * NE :]
            )

            # V[s] = (x @ Wv^T)[s-chunk] + bv -> [128 s, 1024 e] bf16
            for s in range(NSK):
                vs = v_pool.tile([P, D], BF)
                pss = [psp.tile([P, 512], F32, tag="ps", name="ps") for _ in range(2)]
                for d in range(ND):
                    for nt in range(2):
                        nc.tensor.matmul(
                            pss[nt][:],
                            XT[d][:, s * P : (s + 1) * P],
                            WV[d][:, nt * 512 : (nt + 1) * 512],
                            start=(d == 0),
                            stop=(d == ND - 1),
                        )
                for nt in range(2):
                    nc.vector.tensor_tensor(
                        vs[:, nt * 512 : (nt + 1) * 512],
                        pss[nt][:],
                        bv[:, nt * 512 : (nt + 1) * 512],
                        op=Alu.add,
                    )
                V.append(vs)

        # ---------------- phase B: attention + output projection ----------------
        # 3-stage software pipeline over query blocks (big blocks first):
        #   S(t): score matmuls + mask + softmax + DMA-transpose of weights
        #   A(t): attended matmuls + rinv-scaled evict + DMA-transpose
        #   O(t): output projection + bias + store
        # PE program order S(i);A(i-1);O(i-2) keeps the PE dense while
        # softmax/DMA latencies of a block hide under the next block's
        # score matmuls.
        with ExitStack() as bctx:
            sp = bctx.enter_context(tc.tile_pool(name="s_sb", bufs=2))
            wp = bctx.enter_context(tc.tile_pool(name="w_sb", bufs=2))
            wtp = bctx.enter_context(tc.tile_pool(name="wt_sb", bufs=3))
            outp = bctx.enter_context(tc.tile_pool(name="out_sb", bufs=2))
            stat = bctx.enter_context(tc.tile_pool(name="stat", bufs=3))
            ps_s = bctx.enter_context(tc.tile_pool(name="ps_s", bufs=2, space="PSUM"))
            ps_t = bctx.enter_context(tc.tile_pool(name="ps_t", bufs=2, space="PSUM"))
            ps_a = bctx.enter_context(tc.tile_pool(name="ps_a", bufs=3, space="PSUM"))

            def emit_scores(t):
                cov = COVS[t]
                s_sb = sp.tile([P, cov], F32, tag="s")
                # mask term: (k > q) * -1e30, written into s_sb
                nc.vector.tensor_scalar(
                    s_sb[:],
                    kpos[:, :cov],
                    qpos[:, t : t + 1],
                    NEG_BIG,
                    op0=Alu.is_gt,
                    op1=Alu.mult,
                )
                for off, wdt in _score_tiles(cov):
                    ps = ps_s.tile([P, wdt], F32, tag="ps_s")
                    for e in range(NE):
                        nc.tensor.matmul(
                            ps[:],
                            QT[e][:, t * P : (t + 1) * P],
                            XT[e][:, off : off + wdt],
                            start=(e == 0),
                            stop=(e == NE - 1),
                        )
                    nc.vector.tensor_tensor(
                        s_sb[:, off : off + wdt],
                        ps[:],
                        s_sb[:, off : off + wdt],
                        op=Alu.add,
                    )
                negm = stat.tile([P, 1], F32, tag="negm")
                nc.vector.tensor_reduce(
                    negm[:], s_sb[:], axis=X, op=Alu.max, negate=True
                )
                negm32 = stat.tile([P, 1], F32, tag="negm32")
                nc.vector.tensor_scalar_mul(negm32[:], negm[:], float(SCALE))
                w_sb = wp.tile([P, cov], BF, tag="w")
                lsum = stat.tile([P, 1], F32, tag="lsum")
                nc.scalar.activation(
                    w_sb[:],
                    s_sb[:],
                    AF.Exp,
                    bias=negm32[:],
                    scale=float(SCALE),
                    accum_out=lsum[:],
                )
                rinv = stat.tile([P, 1], F32, tag="rinv")
                nc.vector.reciprocal(rinv[:], lsum[:])
                # weight transposes on PE (matmul transpose mode)
                K = cov // P
                wT = wtp.tile([P, cov], BF, tag="wt")
                for k in range(K):
                    pt = ps_t.tile([P, P], BF, tag="pt")
                    nc.tensor.transpose(pt[:], w_sb[:, k * P : (k + 1) * P], ident[:])
                    nc.vector.tensor_copy(wT[:, k * P : (k + 1) * P], pt[:])
                return {"t": t, "wT": wT, "rinv": rinv}

            def emit_attend(st):
                t = st["t"]
                cov = COVS[t]
                K = cov // P
                wT, rinv = st["wT"], st["rinv"]
                outsb = outp.tile([P, D], F32, tag="o")
                for nt in range(2):
                    pa = ps_a.tile([P, 512], F32, tag="pa")
                    for k in range(K):
                        nc.tensor.matmul(
                            pa[:],
                            wT[:, k * P : (k + 1) * P],
                            V[k][:, nt * 512 : (nt + 1) * 512],
                            start=(k == 0),
                            stop=(k == K - 1),
                        )
                    # out = psum * rinv (softmax normalize) then + bvo
                    nc.scalar.activation(
                        outsb[:, nt * 512 : (nt + 1) * 512],
                        pa[:],
                        AF.Copy,
                        bias=0.0,
                        scale=rinv[:],
                    )
                    nc.vector.tensor_tensor(
                        outsb[:, nt * 512 : (nt + 1) * 512],
                        outsb[:, nt * 512 : (nt + 1) * 512],
                        bo[:, nt * 512 : (nt + 1) * 512],
                        op=Alu.add,
                    )
                    nc.sync.dma_start(
                        out_d[t][:, nt * 512 : (nt + 1) * 512],
                        outsb[:, nt * 512 : (nt + 1) * 512],
                    )

            order = list(range(NQB - 1, -1, -1))  # big blocks first
            states = []
            for i, t in enumerate(order):
                states.append(emit_scores(t))
                if i >= 1:
                    emit_attend(states[i - 1])
            emit_attend(states[-1])


def build_nc():
    """Build + compile the SPMD Bass program (cached)."""
    global _NC
    if _NC is not None:
        return _NC
    from concourse import bacc, mybir
    import concourse.tile as tile

    BF = mybir.dt.bfloat16
    F32 = mybir.dt.float32

    nc = bacc.Bacc(
        "TRN2", target_bir_lowering=False, debug=False, enable_asserts=False
    )
    dr = {}

    def din(name, shape, dt):
        dr[name] = nc.dram_tensor(name, shape, dt, kind="ExternalInput").ap()

    din("xT", (ND, P, S), BF)
    din("wqT", (NE, P, ND * P), BF)
    din("wvT", (ND, P, D), BF)
    din("ident", (P, P), BF)
    din("cpak", (P, CPAK), F32)
    out_d = nc.dram_tensor("out_c", (NQB, P, D), F32, kind="ExternalOutput").ap()

    with tile.TileContext(nc) as tc:
        _emit(nc, tc, dr, out_d)
    nc.compile()
    _NC = nc
    return nc


def make_in_maps(x, Wq, bq, Wk, bk, Wv, bv, Wo, bo):
    """Host-side sharding: per-core input dicts (bf16 compute operands)."""
    bf16 = ml_dtypes.bfloat16
    f32 = np.float32

    def chunkg(WT):  # [1024,1024] -> (NE,128,ND*128): [e][p][d*128+c]
        return (
            np.ascontiguousarray(
                WT.reshape(ND, P, NE, P).transpose(2, 1, 0, 3)
            ).reshape(NE, P, ND * P)
        ).astype(bf16)

    # host-fused weights (f32 GEMMs, exact up to fp32):
    #   scores = (x Wq^T)(x Wk^T)^T = x (Wq^T Wk) x^T       -> Wqk
    #   out    = softmax(..) (x Wv^T) Wo^T = softmax(..) x (Wo Wv)^T
    # so K and the output projection never materialize on-chip.
    # Requires bq = bk = 0 (guaranteed by the problem spec).
    Wqk = Wq.T.astype(np.float32) @ Wk.astype(np.float32)  # [d1, d2]
    Wvo = Wo.astype(np.float32) @ Wv.astype(np.float32)  # [e, d]
    wq_c = chunkg(Wqk)
    wv_c = np.ascontiguousarray(Wvo.T).reshape(ND, P, D).astype(bf16)
    bvo = Wo.astype(np.float32) @ bv.astype(np.float32) + bo.astype(np.float32)
    bq_t = np.ascontiguousarray(bq.reshape(NE, P).T).astype(f32)
    bk_t = np.ascontiguousarray(bk.reshape(NE, P).T).astype(f32)
    bv_b = np.zeros((P, D), f32)  # bv folded into bvo
    bo_b = np.broadcast_to(bvo, (P, D))
    ident = np.eye(P, dtype=bf16)

    in_maps = []
    for c in range(8):
        b, j = c // 2, c % 2
        blocks = J_BLOCKS[j]
        other = J_BLOCKS[1 - j]
        # key permutation: slot t holds [my block t | peer block t], so
        # this core's queries are columns [256t, 256t+128) and the first
        # 256(t+1) columns cover every true key <= any query in slot t
        perm = np.concatenate(
            [
                np.r_[P * blocks[t] : P * (blocks[t] + 1),
                      P * other[t] : P * (other[t] + 1)]
                for t in range(NQB)
            ]
        )
        xTb = np.ascontiguousarray(x[b].T[:, perm])  # [D, S] permuted keys
        qpos = (np.array(blocks, dtype=f32) * P)[None, :] + np.arange(
            P, dtype=f32
        )[:, None]
        kpos = np.broadcast_to(perm.astype(f32), (P, S))
        cpak = np.concatenate([qpos, bq_t, bk_t, kpos, bv_b, bo_b], axis=1)
        assert cpak.shape == (P, CPAK)
        in_maps.append(
            {
                "xT": xTb.reshape(ND, P, S).astype(bf16),
                "wqT": wq_c,
                "wvT": wv_c,
                "cpak": np.ascontiguousarray(cpak.astype(f32)),
                "ident": ident,
            }
        )
    return in_maps


def assemble_out(results):
    out = np.empty((B, S, D), dtype=np.float32)
    for c in range(8):
        b, j = c // 2, c % 2
        blocks = J_BLOCKS[j]
        oc = results[c]["out_c"]  # (8, 128, 1024)
        for t, g in enumerate(blocks):
            out[b, P * g : P * (g + 1), :] = oc[t]
    return out


def kernel(x, Wq, bq, Wk, bk, Wv, bv, Wo, bo):
    from concourse.bass_utils import run_bass_kernel_spmd

    nc = build_nc()
    in_maps = make_in_maps(x, Wq, bq, Wk, bk, Wv, bv, Wo, bo)
    res = run_bass_kernel_spmd(nc, in_maps, core_ids=list(range(8)))
    return assemble_out(res.results)
